# revision 2
# baseline (speedup 1.0000x reference)
"""Trainium2 Bass kernel for nn_DictConv2d (FISTA convolutional sparse coding).

v6: fp8-DoubleRow conv_T with stacked-psum output.

Reference (per sample):
    Wn  = W / ||W||_F per filter                 (128, 64, 3, 3)
    c_1 = relu(MU*conv(x, Wn) - thr); y_1 = c_1
    5x:  c_{k+1} = relu(y_k + MU*conv(x - conv_T(y_k), Wn) - thr)
         y_{k+1} = (1+mu_k) c_{k+1} - mu_k c_k
    return c_6

u-form per iteration: u = conv_T(y); c = relu(z - MU*conv(u) - thr), z = y+b,
b = MU*conv(x) precomputed once.

Mapping (2 samples/core, 8 cores, data parallel over batch):

* conv_T: fp8 DoubleRow on y8 (fp8 image, pitch 64). Output pairs
  (R, R+1) stacked in psum partitions (0-63 = u row R, 64-127 = row R+1);
  per chunk of 7 pairs: 6 DR instrs (2 j-phases x 3 dx), activation AP
  [[64,2],[128,7],[1,56]] (ktile = adjacent y rows), psum [128, 392]
  packed. Eviction: 1 ACT copy -> u8 odd stacks + 2 SBUF u8->u8
  partition-shift DMAs fill the even stacks (u rows are stored twice in
  the stacked layout; the second copy is a DMA, not a recompute).

* forward conv: fp8 DR on stacked u8 as before, but with a 3-free-dim
  activation AP [[128,2],[64,8],[1,56]] that skips the 8 pad columns
  per row: psum is 448 packed (no junk), 3 DR per 8-row chunk. The z
  injection (idn fp16 matmul, psum-contiguous write) OPENS each bank
  with start=True; DR taps accumulate with start=False.

* epilogues: momentum as TS (yp = s*c_new) + in-place TT
  (yp -= c_prev) on DVE (2x 16-bit paths, cheaper than 1x STT),
  z = yp + b (TT), y8 = cast(yp) (DVE tensor_copy fp16->fp8), all in 2
  row-bands emitted mid-forward-pass for pipelining.

* weights: fwd + conv_T taps e4m3 (fwd scaled 2^7, conv_T scaled 2^8);
  init taps fp16 (b precision is the error-budget-critical path:
  fp8 init measured 3.5e-2 in numsim vs 1.4e-2 for this config).
"""

import math
import sys

sys.path.insert(0, "/opt/trn_rl_repo")

import numpy as np
import ml_dtypes

import concourse.bass as bass
import concourse.tile as tile
from concourse import mybir
from concourse import bass_utils
from concourse.vector_clock import ScopedClock

F32 = mybir.dt.float32
BF16 = mybir.dt.bfloat16
FP16 = mybir.dt.float16
FP8 = mybir.dt.float8e4
AF = mybir.ActivationFunctionType
ALU = mybir.AluOpType
DRM = mybir.MatmulPerfMode.DoubleRow

MU = 0.1
THR = MU * 0.1          # mu * lambda
N_ITERS = 5
H = W_ = 56
P58 = 58                # state pitch (fp16 padded images, image at (1,1))
P64 = 64                # u/y pitch (fp8, image rows 1..56, u cols 2..57, y cols 1..56)
NP58 = P58 * P58        # 3364
NP64 = P58 * P64        # 3712 (58 rows x 64)
SL = 512                # slack
NPIX = H * W_           # 3136
NG = 8                  # fwd output rows per chunk
NCHUNK = H // NG        # 7
NPAIR = 7               # conv_T pairs per chunk
NCT = 4                 # conv_T chunks (4 x 7 pairs = 28 pairs = 56 rows)
WS = 128.0              # fwd weight scale 2^7
SCT = 256.0             # conv_T weight scale 2^8
NCORES = 8
SPC = 2
BANDP = 29 * W_         # packed flat band split (image rows 1-29)


def _fista_consts():
    t = 1.0
    mu = []
    for _ in range(N_ITERS):
        t_next = (1.0 + math.sqrt(1.0 + 4.0 * t * t)) / 2.0
        mu.append((t - 1.0) / t_next)
        t = t_next
    alpha = [mu[1], mu[2], mu[3], 1.0, 1.0]
    s = [None, (1.0 + mu[1]) / alpha[1], (1.0 + mu[2]) / alpha[2],
         (1.0 + mu[3]) / alpha[3], None]
    inv_a0 = 1.0 / alpha[0]
    return mu, alpha, s, inv_a0


# --- walrus sync-wait workarounds (same as the bf16 baseline) --------------
def _split_drain_and_barrier(self, tick_clock, wait_clock):
    nc = self.nc
    probe = nc.sync.nop()
    wait_clock.add_sem_waits(probe.ins, ScopedClock({None: tick_clock.global_clock}))
    ow = list(probe.ins.sync_info.on_wait) if probe.ins.sync_info else []
    probe.ins.sync_info = mybir.SyncInfo(on_wait=ow[:1], on_update=[])
    for w in ow[1:]:
        nop = nc.sync.nop()
        nop.ins.sync_info = mybir.SyncInfo(on_wait=[w], on_update=[])
    nc.sync.drain()
    nc.all_engine_barrier()
    assert self.sems is not None
    popped = nc._tile_sem_poison_stack.pop()
    assert popped is self._sem_poison
    nc.clear_and_free_semaphores(list(self.sems.allocated().values()))
    nc.all_engine_barrier()


tile.TileContext._drain_and_barrier = _split_drain_and_barrier

_WAIT_LIMIT = 1


def _hoist_excess_waits(nc):
    for fn in nc.m.functions:
        for blk in fn.blocks:
            insts = list(blk.instructions)
            out = []
            changed = False
            for inst in insts:
                si = inst.sync_info
                if si is not None and si.on_wait and len(si.on_wait) > _WAIT_LIMIT:
                    waits = list(si.on_wait)
                    keep = waits[-_WAIT_LIMIT:]
                    for w in waits[:-_WAIT_LIMIT]:
                        nop = mybir.InstNoOp(
                            name=nc.get_next_instruction_name(),
                            engine=inst.engine,
                            bass_nofuse=True,
                            sync_info=mybir.SyncInfo(on_wait=[w], on_update=[]),
                        )
                        nc.register_instruction(nop)
                        out.append(nop)
                    inst.sync_info = mybir.SyncInfo(
                        on_wait=keep, on_update=list(si.on_update)
                    )
                    changed = True
                out.append(inst)
            if changed:
                blk.instructions = out


def _ap(t, offset, dims):
    """Manual AP: keep the partition dim, set free dims [[step, num], ...]."""
    a = t.copy() if isinstance(t, bass.AP) else t.ap()
    a.ap = a.ap[:1] + dims
    a.offset = a.offset + offset
    return a


# ---------------------------------------------------------------------------
def _build_program():
    mu, alpha, s_k, inv_a0 = _fista_consts()

    nc = bass.Bass("TRN2", debug=False, num_devices=NCORES)

    x_d = nc.dram_tensor("x16", [SPC, 64, NP58 + P58], FP16, kind="ExternalInput")
    wfp_d = nc.dram_tensor("wfp16", [128, 3 * 128], FP16, kind="ExternalInput")
    wfs_d = nc.dram_tensor("wfs16", [128, 3 * 128], FP16, kind="ExternalInput")
    wd_d = nc.dram_tensor("wd8", [128, 3 * 256], FP8, kind="ExternalInput")
    wct_d = nc.dram_tensor("wctdr8", [128, 6 * 256], FP8, kind="ExternalInput")
    idn_d = nc.dram_tensor("idns", [128, 128], FP16, kind="ExternalInput")
    out_d = nc.dram_tensor("out", [SPC, 128, NPIX], FP16, kind="ExternalOutput")

    with tile.TileContext(nc) as tc:
        with (
            tc.tile_pool(name="pers", bufs=1) as pers,
            tc.tile_pool(name="psum", bufs=3, space="PSUM") as psum,
        ):
            warm = pers.tile([128, 128], FP16, tag="warm")
            nc.gpsimd.memset(warm, 0.0)
            zro = pers.tile([128, NG * W_], FP16, tag="zro")
            nc.gpsimd.memset(zro, 0.0)
            for v in {-THR} | {-a * THR for a in alpha}:
                ct = pers.tile([128, 1], F32, tag=f"cst{v}")
                nc.vector.memset(ct, v)
                nc.const_aps.aps[(F32, v)] = ct
            wfp = pers.tile([128, 3 * 128], FP16, tag="wfp")
            wfs = pers.tile([128, 3 * 128], FP16, tag="wfs")
            wd = pers.tile([128, 3 * 256], FP8, tag="wd")
            wct = pers.tile([128, 6 * 256], FP8, tag="wct")
            idn = pers.tile([128, 128], FP16, tag="idn")
            nc.scalar.dma_start(out=wfp, in_=wfp_d.ap())
            nc.sync.dma_start(out=wfs, in_=wfs_d.ap())

            x16, u8, y8, yp16, z16, b16, cbuf = [], [], [], [], [], [], []
            for s in range(SPC):
                xb = pers.tile([128, NP58 + P58], FP16, tag=f"xb{s}", name=f"xb{s}")
                ub = pers.tile([128, NP64], FP8, tag=f"ub{s}", name=f"ub{s}")
                yq = pers.tile([128, NP64], FP8, tag=f"yq{s}", name=f"yq{s}")
                # packed (pitch-56, no pads) fp16 state: ACT/inject fully
                # contiguous, DVE flats 7% smaller, no pad memsets
                yb = pers.tile([128, NPIX], FP16, tag=f"yb{s}", name=f"yb{s}")
                zb = pers.tile([128, NPIX], FP16, tag=f"zb{s}", name=f"zb{s}")
                bb = pers.tile([128, NPIX], FP16, tag=f"bb{s}", name=f"bb{s}")
                ca = pers.tile([128, NPIX], FP16, tag=f"ca{s}", name=f"ca{s}")
                cb = pers.tile([128, NPIX], FP16, tag=f"cb{s}", name=f"cb{s}")
                x16.append(xb); u8.append(ub); y8.append(yq); yp16.append(yb)
                z16.append(zb); b16.append(bb); cbuf.append((ca, cb))
                # u8/y8 pad regions read by taps (image area is rewritten
                # every iteration; fp8 garbage elsewhere could be NaN)
                u0 = ub.bitcast(mybir.dt.uint8)
                y0 = yq.bitcast(mybir.dt.uint8)
                nc.vector.memset(_ap(u0, 0, [[1, P64]]), 0)            # row 0
                nc.vector.memset(_ap(u0, 57 * P64, [[1, P64]]), 0)     # row 57
                # stack-56 upper half (row 57) is never written by
                # evict/swaps; fp8 garbage there can be NaN (NaN*0=NaN)
                nc.vector.memset(_ap(u0[64:128], 56 * P64, [[1, P64]]), 0)
                nc.vector.memset(_ap(u0, P64 + 1, [[P64, 56], [1, 1]]), 0)
                nc.vector.memset(_ap(u0, P64 + 58, [[P64, 56], [1, 1]]), 0)
                nc.vector.memset(_ap(y0, 0, [[1, P64]]), 0)
                nc.vector.memset(_ap(y0, 57 * P64, [[1, P64]]), 0)
                nc.vector.memset(_ap(y0, P64, [[P64, 56], [1, 1]]), 0)
                nc.vector.memset(_ap(y0, P64 + 57, [[P64, 56], [1, 1]]), 0)

            # x streamed in row-bands, samples interleaved, 4 queues: init
            # chunk c of each sample can start as soon as its band lands
            qs = [nc.sync, nc.gpsimd, nc.scalar]
            qi = 0
            for b0, b1 in ((0, 11), (11, 22), (22, 35), (35, 46), (46, 58)):
                f0, f1 = b0 * P58, b1 * P58
                for s in range(SPC):
                    qs[qi % 3].dma_start(out=x16[s][0:64, f0:f1],
                                         in_=x_d.ap()[s, :, f0:f1])
                    qs[(qi + 1) % 3].dma_start(
                        out=x16[s][64:128, f0:f1],
                        in_=x_d.ap()[s, :, P58 + f0:P58 + f1])
                    qi += 2
            nc.gpsimd.dma_start(out=wd, in_=wd_d.ap())
            nc.scalar.dma_start(out=wct, in_=wct_d.ap())
            nc.scalar.dma_start(out=idn, in_=idn_d.ap())

            # HAM warmup: keep the PE busy during the x DMA-in so the clock
            # gate opens (1.2 -> 2.4 GHz) before the first real matmul
            pw = psum.tile([128, 128], F32, tag="pc", name="pw", bufs=3)
            for wi in range(34):
                nc.tensor.matmul(pw, warm, warm, start=True,
                                 stop=True).annotate(f"wu{wi}")

            x3 = [t.rearrange("p (r c) -> p r c", c=P58) for t in x16]

            # ---- init: b = MU conv(x) (fp16 taps); c1 = relu(b - thr) -----
            def init_band(s, lo, hi):
                # iter-0 y/z: y1 = c1; z = c1 + b
                nc.vector.tensor_copy(yp16[s][:, lo:hi], cbuf[s][0][:, lo:hi])
                if lo == 0:
                    cast_band(s, 1, 29)
                    cast_band(s, 29, 30)
                else:
                    cast_band(s, 30, 56)
                    cast_band(s, 56, 57)
                nc.vector.tensor_tensor(
                    z16[s][:, lo:hi], yp16[s][:, lo:hi],
                    b16[s][:, lo:hi], ALU.add)

            def cast_band(s, r0, r1):
                # y8 image rows r0..r1) = fp8(yp16 packed rows r0-1..)
                # (even nr keeps the DVE 2x path: 1813ns vs 919ns)
                nr = r1 - r0
                nc.vector.tensor_copy(
                    _ap(y8[s], P64 * r0 + 1, [[P64, nr], [1, W_]]),
                    _ap(yp16[s], W_ * (r0 - 1),
                        [[W_, nr], [1, W_]])).annotate(f"cy{s}.{r0}")

            for s in range(SPC):
                for c in range(NCHUNK):
                    g0 = c * NG
                    pi = psum.tile([128, NG * W_], F32, tag="pf", name="pi",
                                   bufs=5)
                    first = True
                    for dx in range(3):
                        nc.tensor.matmul(
                            pi, wfp[:, dx * 128:(dx + 1) * 128],
                            x3[s][:, g0:g0 + NG, dx:dx + W_],
                            start=first, stop=False)
                        first = False
                    for dx in range(3):
                        nc.tensor.matmul(
                            pi, wfs[:, dx * 128:(dx + 1) * 128],
                            x3[s][:, g0 + 2:g0 + 2 + NG, dx:dx + W_],
                            start=False, stop=(dx == 2))
                    f0 = g0 * W_
                    # b holds b - thr so every later relu is bias-free
                    nc.scalar.activation(
                        b16[s][:, f0:f0 + NG * W_], pi, AF.Copy,
                        bias=-THR).annotate(f"ib{s}.{c}")
                    nc.vector.tensor_scalar(
                        cbuf[s][0][:, f0:f0 + NG * W_], pi,
                        THR, 0.0, ALU.subtract,
                        ALU.max).annotate(f"ic{s}.{c}")
                    if c == 3:
                        init_band(s, 0, BANDP)
                init_band(s, BANDP, NPIX)

            # ---- 5 FISTA iterations --------------------------------------
            def emit_convt(s):
                # 4 chunks of 7 output pairs (R0, R0+1), R0 = 1 + 14*cc
                for cc in range(NCT):
                    R0 = 1 + 14 * cc
                    pc = psum.tile([128, NPAIR * W_], F32, tag="pc", name="pc",
                                   bufs=3)
                    first = True
                    for phi in range(2):
                        for dx in range(3):
                            q = phi * 3 + dx
                            nc.tensor.matmul(
                                pc, _ap(wct, q * 256, [[128, 2], [1, 128]]),
                                _ap(y8[s], P64 * (R0 - 1 + 2 * phi) + dx,
                                    [[P64, 2], [2 * P64, NPAIR], [1, W_]]),
                                start=first, stop=(phi == 1 and dx == 2),
                                perf_mode=DRM).annotate(f"ct{s}.{cc}.{q}")
                            first = False
                    # direct eviction: odd stacks (R0, R0+2, ...) all 128 parts
                    nc.scalar.activation(
                        _ap(u8[s], P64 * R0 + 2, [[2 * P64, NPAIR], [1, W_]]),
                        pc, AF.Copy, scale=1.0 / SCT).annotate(f"ev{s}.{cc}")
                    # even stacks via SBUF partition-shift DMAs:
                    # lower half of stack q+1 = upper half of stack q (row q+1)
                    nc.sync.dma_start(
                        out=_ap(u8[s][0:64], P64 * (R0 + 1) + 2,
                                [[2 * P64, NPAIR], [1, W_]]),
                        in_=_ap(u8[s][64:128], P64 * R0 + 2,
                                [[2 * P64, NPAIR], [1, W_]])).annotate(f"swA{s}.{cc}")
                    # upper half of stack q-1 = lower half of stack q (row q)
                    eng = nc.sync if cc % 2 == 0 else nc.gpsimd
                    eng.dma_start(
                        out=_ap(u8[s][64:128], P64 * (R0 - 1) + 2,
                                [[2 * P64, NPAIR], [1, W_]]),
                        in_=_ap(u8[s][0:64], P64 * R0 + 2,
                                [[2 * P64, NPAIR], [1, W_]])).annotate(f"swB{s}.{cc}")

            def emit_fwd_chunk(s, k, c):
                    a = alpha[k]
                    last = k == N_ITERS - 1
                    R = c * NG
                    pf = psum.tile([128, NG * W_], F32, tag="pf", name="pf",
                                   bufs=5)
                    # z-tap opens the bank (psum = WS * z); fully contiguous
                    nc.tensor.matmul(
                        pf, idn, z16[s][:, R * W_:(R + NG) * W_],
                        start=True, stop=False,
                        skip_group_check=True).annotate(f"zi{s}.{c}")
                    for t3 in range(3):
                        nc.tensor.matmul(
                            pf, _ap(wd, t3 * 256, [[128, 2], [1, 128]]),
                            _ap(u8[s], P64 * R + 1 + t3,
                                [[2 * P64, 2], [P64, NG], [1, W_]]),
                            start=False, stop=(t3 == 2), perf_mode=DRM,
                            skip_group_check=True).annotate(f"fw{s}.{c}.{t3}")
                    if last:
                        ob = pers.tile([128, NG * W_], FP16, tag="ob",
                                       name="ob", bufs=14)
                        if s == 0:
                            nc.vector.scalar_tensor_tensor(
                                ob, pf, 1.0 / WS, zro, ALU.mult,
                                ALU.max).annotate(f"ro{s}.{c}")
                        else:
                            nc.scalar.activation(
                                ob, pf, AF.Relu,
                                scale=1.0 / WS).annotate(f"ro{s}.{c}")
                        if c < 4:
                            oeng = (nc.sync, nc.gpsimd, nc.scalar)[(2 * c + s) % 3]
                        else:
                            oeng = nc.sync if (c + s) % 2 == 0 else nc.scalar
                        if c == NCHUNK - 1:
                            oeng2 = nc.scalar if (c + s) % 2 == 0 else nc.sync
                            half = NG * W_ // 2
                            oeng.dma_start(
                                out=out_d.ap()[s, :, R * W_:R * W_ + half],
                                in_=ob[:, 0:half])
                            oeng2.dma_start(
                                out=out_d.ap()[s, :, R * W_ + half:(R + NG) * W_],
                                in_=ob[:, half:NG * W_])
                        else:
                            oeng.dma_start(
                                out=out_d.ap()[s, :, R * W_:(R + NG) * W_],
                                in_=ob)
                    else:
                        nc.scalar.activation(
                            cbuf[s][(k + 1) % 2][:, R * W_:(R + NG) * W_],
                            pf, AF.Relu,
                            scale=a / WS).annotate(f"re{s}.{c}")
                        if c == 3:
                            emit_tail_band(s, k, 0, BANDP)

            def emit_tail_band(s, k, lo, hi):
                """yp16/z16 for iteration k+1 (one flat band)."""
                cdst, csrc = cbuf[s][(k + 1) % 2], cbuf[s][k % 2]
                if k == 0:
                    nc.vector.tensor_scalar_mul(
                        yp16[s][:, lo:hi], cdst[:, lo:hi],
                        inv_a0).annotate(f"tm{s}.{lo > 0}")
                else:
                    # y = s_k * c_new - c_prev  (TS then in-place TT: both 2x)
                    nc.vector.tensor_scalar_mul(
                        yp16[s][:, lo:hi], cdst[:, lo:hi],
                        s_k[k]).annotate(f"ts{s}.{lo > 0}")
                    nc.vector.tensor_tensor(
                        yp16[s][:, lo:hi], yp16[s][:, lo:hi],
                        csrc[:, lo:hi], ALU.subtract).annotate(f"tm{s}.{lo > 0}")
                # cast before the z-add: next iter's convT consumes y8 first
                if lo == 0:
                    cast_band(s, 1, 29)
                    cast_band(s, 29, 30)
                else:
                    cast_band(s, 30, 56)
                    cast_band(s, 56, 57)
                nc.vector.tensor_tensor(
                    z16[s][:, lo:hi], yp16[s][:, lo:hi],
                    b16[s][:, lo:hi], ALU.add).annotate(f"tz{s}.{lo > 0}")

            for k in range(N_ITERS):
                emit_convt(0)
                emit_convt(1)
                if k < N_ITERS - 1:
                    for s in range(SPC):
                        for c in range(NCHUNK):
                            emit_fwd_chunk(s, k, c)
                        emit_tail_band(s, k, BANDP, NPIX)
                else:
                    # last iteration: interleave samples so output DMA
                    # transfers spread across the whole window
                    for c in range(NCHUNK):
                        for s in range(SPC):
                            emit_fwd_chunk(s, k, c)

    _hoist_excess_waits(nc)
    return nc


# ---------------------------------------------------------------------------
def _host_prep(x, W):
    x = np.asarray(x, dtype=np.float32)
    W = np.asarray(W, dtype=np.float32)
    Wn = W / np.sqrt((W * W).sum(axis=(1, 2, 3), keepdims=True) + 1e-12)

    f16 = ml_dtypes.float16 if hasattr(ml_dtypes, "float16") else np.float16
    f8 = ml_dtypes.float8_e4m3fn

    # init fwd taps (fp16): stacked pairs + zero-padded singles, MU folded
    wfp = np.empty((128, 3 * 128), dtype=np.float32)
    wfs = np.zeros((128, 3 * 128), dtype=np.float32)
    for dx in range(3):
        wfp[0:64, dx * 128:(dx + 1) * 128] = MU * Wn[:, :, 0, dx].T
        wfp[64:128, dx * 128:(dx + 1) * 128] = MU * Wn[:, :, 1, dx].T
        wfs[0:64, dx * 128:(dx + 1) * 128] = MU * Wn[:, :, 2, dx].T

    # fwd DR taps (fp8): 3 taps, each [128, 2, 128]; ktile0 = stacked pair
    # (-MU*WS scaled), ktile1 = dy=2 single (rows 64-127 zero)
    wd = np.zeros((128, 3 * 256), dtype=np.float32)
    for dx in range(3):
        base = dx * 256
        wd[0:64, base:base + 128] = -MU * WS * Wn[:, :, 0, dx].T
        wd[64:128, base:base + 128] = -MU * WS * Wn[:, :, 1, dx].T
        wd[0:64, base + 128:base + 256] = -MU * WS * Wn[:, :, 2, dx].T

    # conv_T DR taps (fp8): 6 blocks (phi-major, then dx), each [128, 2, 128].
    # ktile (phi, i): weight col m<64 -> psum lower (u row R), tap dy=2*phi+i;
    # col m>=64 -> psum upper (u row R+1), tap dy=2*phi+i-1.
    # Wct_tap[dy, dx] = Wn[:, :, 2-dy, 2-dx] (conv_transpose flip).
    wct = np.zeros((128, 6 * 256), dtype=np.float32)
    for phi in range(2):
        for dx in range(3):
            base = (phi * 3 + dx) * 256
            for i in range(2):
                dy_lo = 2 * phi + i
                dy_hi = dy_lo - 1
                if 0 <= dy_lo <= 2:
                    wct[:, base + i * 128:base + i * 128 + 64] = \
                        SCT * Wn[:, :, 2 - dy_lo, 2 - dx]
                if 0 <= dy_hi <= 2:
                    wct[:, base + i * 128 + 64:base + (i + 1) * 128] = \
                        SCT * Wn[:, :, 2 - dy_hi, 2 - dx]

    idn = np.eye(128, dtype=np.float32) * WS

    n = x.shape[0]
    xpad = np.zeros((n, 64, P58, P58), dtype=np.float32)
    xpad[:, :, 1:1 + H, 1:1 + W_] = x
    xpad = xpad.reshape(n, 64, NP58)
    xpad = np.concatenate(
        [xpad, np.zeros((n, 64, P58), dtype=np.float32)], axis=2)

    shared = {
        "wfp16": wfp.astype(f16),
        "wfs16": wfs.astype(f16),
        "wd8": np.clip(wd, -240, 240).astype(f8),
        "wctdr8": np.clip(wct, -240, 240).astype(f8),
        "idns": idn.astype(f16),
    }
    x16 = xpad.astype(f16)
    in_maps = []
    for core in range(NCORES):
        slb = x16[core * SPC:(core + 1) * SPC]
        in_maps.append({"x16": np.ascontiguousarray(slb), **shared})
    return in_maps


_CACHED_NC = None


def _get_nc():
    global _CACHED_NC
    if _CACHED_NC is None:
        _CACHED_NC = _build_program()
    return _CACHED_NC


def _run(x, W, **kwargs):
    in_maps = _host_prep(x, W)
    nc = _get_nc()
    res = bass_utils.run_bass_kernel_spmd(
        nc, in_maps, core_ids=list(range(NCORES)), **kwargs)
    outs = [res.results[i]["out"].astype(np.float32).reshape(SPC, 128, H, W_)
            for i in range(NCORES)]
    full = np.concatenate(outs, axis=0)
    return full, res


def kernel(x, W):
    out, _ = _run(x, W)
    return out


def kernel_profiled(x, W, tmpdir=None):
    _install_ntff_hook()
    out, res = _run(x, W, trace=True, tmpdir=tmpdir)
    return out, res


def _install_ntff_hook():
    """Register the axon NTFF profiling hook (the image's antenv lacks
    axon_hooks; drive the stable C ABI in libaxon_pjrt.so directly)."""
    import contextlib
    import ctypes
    import types

    try:
        from antenv.axon_hooks import get_axon_ntff_profile_hook  # noqa: F401
        return
    except ImportError:
        pass

    so_path = "/opt/axon/libaxon_pjrt.so"
    lib = ctypes.CDLL(so_path)
    if not hasattr(lib, "axon_start_nrt_profile"):
        return
    lib.axon_start_nrt_profile.argtypes = [
        ctypes.POINTER(ctypes.c_int64), ctypes.c_size_t]
    lib.axon_start_nrt_profile.restype = ctypes.c_int64
    lib.axon_stop_nrt_profile.argtypes = [ctypes.c_char_p]
    lib.axon_stop_nrt_profile.restype = ctypes.c_int64

    @contextlib.contextmanager
    def _hook(output_dir, device_ids):
        import jax
        jax.devices()
        if device_ids:
            ids = (ctypes.c_int64 * len(device_ids))(*device_ids)
            rc = lib.axon_start_nrt_profile(ids, len(device_ids))
        else:
            rc = lib.axon_start_nrt_profile(None, 0)
        if rc != 0:
            raise RuntimeError(f"axon_start_nrt_profile rc={rc}")
        try:
            yield
        finally:
            n = lib.axon_stop_nrt_profile(str(output_dir).encode())
            if n < 0:
                raise RuntimeError(f"axon_stop_nrt_profile rc={n}")
            if n == 0:
                print("WARNING: NTFF capture wrote no files")

    mod = types.ModuleType("antenv.axon_hooks")
    mod.get_axon_ntff_profile_hook = lambda: _hook
    mod.set_axon_ntff_profile_hook = lambda h: None
    sys.modules["antenv.axon_hooks"] = mod
